# revision 27
# baseline (speedup 1.0000x reference)
"""GQA attention (RoPE + softmax + o_proj) on 8 Trainium2 NeuronCores.

Problem shapes (hardcoded): hidden_states [4, 2048, 2048], 16 q heads,
4 kv heads, head_dim 128, rope cos/sin tables given as inputs.

Sharding: core c -> (batch b = c // 2, q-head half = c % 2).  Each core
computes 8 q heads + their 2 kv heads for one batch and produces a
partial o_proj output [2048, 2048]; the host sums the two halves per
batch (tensor parallel, no device collectives).

All matmuls run in fp16 (1 cycle/row on PE) with fp32 PSUM accumulation.
Key engine-level structure (from perfetto trace analysis):
  - 512-wide moving dims everywhere so LoadStationary (97ns) hides
    behind the 213ns matmul; v is therefore computed TRANSPOSED
    (wv-halves stationary, hs moving 512) at full rate and rotated back
    into [t, d] layout with per-tile DMA crossbar transposes on the
    sync queue -- zero PE/DVE cost (the old [t,d]-direct v projection
    had a 256-wide moving dim and ran at half rate, ~27us lost).
  - RoPE via a partition shuffle (host-permuted head dim) on DVE.
  - scores^T[t, s] with k^T tiles stationary; exp via ScalarE (fused
    1/sqrt(d) scale) reads two PSUM banks per instruction and writes
    P^T fp16 straight to SBUF.
  - phase B software pipeline: stage (h,si) emits its 8 score pairs
    interleaved with the PREVIOUS stage's PV pairs and with o_proj
    matmul bursts of the previous s-block, so PE never idles while
    ScalarE's exp stream (~8us/stage) drains.
  - softmax denominators: four fp16 DVE pairwise adds over P^T down to
    one tile, then a single all-ones stationary matmul (result
    replicated across partitions = pre-broadcast), fast DVE reciprocal,
    fused normalize+cast on the attn PSUM->SBUF copyback.
  - PSUM: score-pair pool (3 bufs x 2 banks) also serves the o_proj and
    denominator accumulators (short-lived allocs in rotation); PV
    accumulators get 2 dedicated banks.
"""

import sys

import numpy as np

B, S, HID = 4, 2048, 2048
NH, NKV, HD = 16, 4, 128
NH_L = 8        # q heads per core
NKV_L = 2       # kv heads per core
GROUP = NH // NKV
P = 128
ST = 512        # s-block (matmul free dim)
NSB = S // ST   # 4 s-blocks
KT = HID // P   # 16 contraction tiles over hidden
TT = S // P     # 16 key/t tiles
SCALE = 1.0 / float(np.sqrt(HD))

_CACHE = {}


def _build():
    if "/opt/trn_rl_repo" not in sys.path:
        sys.path.insert(0, "/opt/trn_rl_repo")
    import concourse.mybir as mybir
    from concourse import bacc
    from concourse.tile import TileContext
    from concourse.tile_rust import add_dep_helper

    dt = mybir.dt
    f16, f32 = dt.float16, dt.float32

    nc = bacc.Bacc("TRN2", target_bir_lowering=False, debug=False, num_devices=8)
    # host-pretiled layouts (see kernel() below)
    hsT = nc.dram_tensor("hsT", [P, NSB, KT, ST], f16, kind="ExternalInput").ap()
    wq = nc.dram_tensor("wq", [P, NH_L, KT, HD], f16, kind="ExternalInput").ap()
    wk = nc.dram_tensor("wk", [P, NKV_L, KT, HD], f16, kind="ExternalInput").ap()
    wv = nc.dram_tensor("wv", [P, KT, NKV_L * HD], f16, kind="ExternalInput").ap()
    wo = nc.dram_tensor("wo", [P, NH_L, HID], f16, kind="ExternalInput").ap()
    cosT = nc.dram_tensor("cosT", [HD, S], f16, kind="ExternalInput").ap()
    sinT = nc.dram_tensor("sinT", [HD, S], f16, kind="ExternalInput").ap()
    out = nc.dram_tensor("out", [S, HID], f32, kind="ExternalOutput").ap()

    EXP = mybir.ActivationFunctionType.Exp

    with TileContext(nc) as tc:
        with (
            tc.tile_pool(name="consts", bufs=1) as consts,
            tc.tile_pool(name="qkv", bufs=1) as qkvp,
        ):
            ones = consts.tile([P, P], f16, tag="ones")
            nc.vector.memset(ones, 1.0)
            # rotate_half as an intra-quadrant partition shuffle (the head
            # dim is host-permuted so +-64 pairs sit 16 apart per quadrant;
            # the sign lives in the pre-negated sin table)
            SHUF = list(range(16, 32)) + list(range(0, 16))

            q_sb = qkvp.tile([P, NH_L, S], f16, tag="q")
            k_sb = qkvp.tile([P, NKV_L, S], f16, tag="k")
            v_sb = qkvp.tile([P, TT, NKV_L * HD], f16, tag="v")

            # ---------------- Phase A: projections + RoPE ----------------
            with (
                tc.tile_pool(name="wqkv", bufs=1) as wp,
                tc.tile_pool(name="trig", bufs=1) as trig,
                tc.tile_pool(name="vT", bufs=1) as vtp,
                tc.tile_pool(name="hs", bufs=2) as hsp,
                tc.tile_pool(name="ropes", bufs=4) as smalls,
                tc.tile_pool(name="psA", bufs=6, space="PSUM") as psA,
            ):
                vT_sb = vtp.tile([P, NKV_L, S], f16, tag="vT")
                # hs block 0 first (its consumers are the head of the program)
                hs_blks = {}
                # first hs block + wv arrive in interleaved chunks so the
                # very first projection group starts after ~0.5MB, not ~3MB
                hs_first = hsp.tile([P, KT, ST], f16, tag="hs")
                wv_sb = wp.tile([P, KT, NKV_L * HD], f16, tag="wv")
                # hs block 0 split across two queues (scalar+gpsimd) so the
                # first projection group is fed at ~2x single-queue rate
                # fine-grained alternating chunks on two dedicated queues so
                # the first vT/k projections are fed smoothly; everything not
                # needed in the first ~15us (wk, cos/sin, wq, wo) is deferred
                # behind hs0 so the critical 3MB (hs block 0 + wv) gets the
                # full ~358 GB/s of per-core HBM bandwidth
                nc.scalar.dma_start(out=hs_first[:, 0:1, :], in_=hsT[:, 0, 0:1, :])
                nc.sync.dma_start(out=wv_sb[:, 0:2, :], in_=wv[:, 0:2, :])
                nc.gpsimd.dma_start(out=hs_first[:, 1:3, :], in_=hsT[:, 0, 1:3, :])
                nc.scalar.dma_start(out=hs_first[:, 3:5, :], in_=hsT[:, 0, 3:5, :])
                nc.sync.dma_start(out=wv_sb[:, 2:9, :], in_=wv[:, 2:9, :])
                nc.gpsimd.dma_start(out=hs_first[:, 5:7, :], in_=hsT[:, 0, 5:7, :])
                nc.scalar.dma_start(out=hs_first[:, 7:9, :], in_=hsT[:, 0, 7:9, :])
                nc.gpsimd.dma_start(out=hs_first[:, 9:11, :], in_=hsT[:, 0, 9:11, :])
                nc.scalar.dma_start(out=hs_first[:, 11:13, :], in_=hsT[:, 0, 11:13, :])
                nc.sync.dma_start(out=wv_sb[:, 9:16, :], in_=wv[:, 9:16, :])
                hs0_dma = nc.gpsimd.dma_start(
                    out=hs_first[:, 13:16, :], in_=hsT[:, 0, 13:16, :]
                )
                hs_blks[0] = hs_first
                hs_dmas = [hs0_dma]

                wk_sb = wp.tile([P, NKV_L, KT, HD], f16, tag="wk")
                wkd = nc.sync.dma_start(out=wk_sb, in_=wk)
                add_dep_helper(
                    wkd.ins, hs0_dma.ins, sync=True, reason="defer wk behind hs0"
                )

                wq_sb = wp.tile([P, NH_L, KT, HD], f16, tag="wq")
                nc.sync.dma_start(out=wq_sb[:, 0, :, :], in_=wq[:, 0, :, :])

                cos_sb = trig.tile([HD, S], f16, tag="cos")
                nc.sync.dma_start(out=cos_sb, in_=cosT)
                sin_sb = trig.tile([HD, S], f16, tag="sin")
                nc.sync.dma_start(out=sin_sb, in_=sinT)
                for h in range(1, NH_L):  # per-head DMAs so early heads land first
                    nc.sync.dma_start(out=wq_sb[:, h, :, :], in_=wq[:, h, :, :])

                # software pipeline: the rot-shuffle + rope combine for one
                # projection is emitted while the NEXT projection's matmul
                # group runs, so PE never waits on the PSUM copyback.
                pending = []

                def rope_flush():
                    qc, s0, dst, dsti = pending.pop(0)
                    rc = smalls.tile([P, ST], f16, tag="rc")
                    nc.vector.stream_shuffle(rc, qc, SHUF)
                    t1 = smalls.tile([P, ST], f16, tag="t1")
                    nc.vector.tensor_mul(t1, qc, cos_sb[:, s0 : s0 + ST])
                    t2 = smalls.tile([P, ST], f16, tag="t2")
                    nc.vector.tensor_mul(t2, rc, sin_sb[:, s0 : s0 + ST])
                    nc.vector.tensor_add(dst[:, dsti, s0 : s0 + ST], t1, t2)

                for si in range(NSB):
                    s0 = si * ST
                    if si in hs_blks:
                        hs_blk = hs_blks[si]
                    else:
                        hs_blk = hsp.tile([P, KT, ST], f16, tag="hs")
                        # gpsimd queue is otherwise idle, so chaining these
                        # issues behind the previous block stalls nothing
                        hd = nc.gpsimd.dma_start(out=hs_blk, in_=hsT[:, si, :, :])
                        add_dep_helper(
                            hd.ins,
                            hs_dmas[-1].ins,
                            sync=True,
                            reason="stagger hs blocks",
                        )
                        hs_dmas.append(hd)

                    def proj(w_slice, dst, dsti):
                        pm = psA.tile([P, ST], f32, tag="ps")
                        for kt in range(KT):
                            nc.tensor.matmul(
                                pm,
                                lhsT=w_slice[:, kt, :],
                                rhs=hs_blk[:, kt, :],
                                start=(kt == 0),
                                stop=(kt == KT - 1),
                            )
                        qc = smalls.tile([P, ST], f16, tag="qc")
                        nc.vector.tensor_copy(qc, pm)
                        pending.append((qc, s0, dst, dsti))

                    # v^T first: needs only hs + the small wv.  Full-rate
                    # (512-wide moving): wv column-half stationary, hs moving.
                    for j in range(NKV_L):
                        pv = psA.tile([P, ST], f32, tag="ps")
                        for kt in range(KT):
                            nc.tensor.matmul(
                                pv,
                                lhsT=wv_sb[:, kt, j * HD : (j + 1) * HD],
                                rhs=hs_blk[:, kt, :],
                                start=(kt == 0),
                                stop=(kt == KT - 1),
                            )
                        nc.scalar.copy(vT_sb[:, j, s0 : s0 + ST], pv)
                    for jk in range(NKV_L):
                        proj(wk_sb[:, jk], k_sb, jk)
                        if len(pending) > 1:
                            rope_flush()
                    # rotate v^T[d, s] back into v[t, d] tiles for PV via the
                    # DMA crossbar transpose (zero PE/DVE cost; the gpsimd
                    # queue is otherwise idle).  Per-tile calls: the dst of
                    # each is a contiguous 128-elem run per partition (3D
                    # strided xbar dsts are a known-wrong path on HW).
                    for sj in range(ST // P):
                        tt = si * (ST // P) + sj
                        for j in range(NKV_L):
                            nc.sync.dma_start_transpose(
                                v_sb[:, tt, j * HD : (j + 1) * HD],
                                vT_sb[:, j, s0 + sj * P : s0 + (sj + 1) * P],
                            )
                    for h in range(NH_L):
                        proj(wq_sb[:, h], q_sb, h)
                        if len(pending) > 1:
                            rope_flush()
                while pending:
                    rope_flush()

            # ---------------- Phase B: attention + interleaved o_proj ------
            with (
                tc.tile_pool(name="wo", bufs=1) as wop,
                tc.tile_pool(name="attn", bufs=1) as ap_,
                tc.tile_pool(name="pblk", bufs=2) as pp,
                tc.tile_pool(name="phalf", bufs=2) as php,
                tc.tile_pool(name="rcps", bufs=4) as rcpp,
                tc.tile_pool(name="outp", bufs=3) as op_,
                tc.tile_pool(name="psc", bufs=3, space="PSUM") as pscp,
                tc.tile_pool(name="pat", bufs=2, space="PSUM") as patp,
            ):
                wo_sb = wop.tile([P, NH_L, HID], f16, tag="wo")
                wod = nc.sync.dma_start(out=wo_sb, in_=wo)
                add_dep_helper(
                    wod.ins, hs0_dma.ins, sync=True, reason="defer wo behind hs0"
                )
                attnT = ap_.tile([P, NH_L, S], f16, tag="attnT")
                QT = TT // 4

                def o_burst(si, sj, ni):
                    """one o_proj psum group: 8 matmuls + copyback + DMA.
                    Copyback on DVE: ScalarE runs at ~98% on the exp stream."""
                    st = si * (ST // P) + sj
                    pot = pscp.tile([P, 2, ST], f32, tag="psc")
                    po = pot[:, 0, :]
                    for ft in range(NH_L):
                        nc.tensor.matmul(
                            po,
                            lhsT=attnT[:, ft, st * P : (st + 1) * P],
                            rhs=wo_sb[:, ft, ni * ST : (ni + 1) * ST],
                            start=(ft == 0),
                            stop=(ft == NH_L - 1),
                        )
                    ob = op_.tile([P, ST], f32, tag="ob")
                    nc.vector.tensor_copy(ob, po)
                    nc.sync.dma_start(
                        out=out[st * P : (st + 1) * P, ni * ST : (ni + 1) * ST],
                        in_=ob,
                    )

                def o_chunks(si):
                    for sj in range(ST // P):
                        for ni in range(HID // ST):
                            yield (si, sj, ni)

                def tree_l1a(pblk):
                    """first tree level over P^T tiles 0-7 (exp groups 0-3)"""
                    ph = php.tile([P, TT // 2, ST], f16, tag="ph")
                    nc.vector.tensor_add(
                        ph[:, 0:4, :], pblk[:, 0:8:2, :], pblk[:, 1:8:2, :]
                    )
                    return ph

                def tree_rest(ph, pblk):
                    nc.vector.tensor_add(
                        ph[:, 4:8, :], pblk[:, 8:16:2, :], pblk[:, 9:16:2, :]
                    )
                    nc.vector.tensor_add(
                        ph[:, 0:8:2, :], ph[:, 0:8:2, :], ph[:, 1:8:2, :]
                    )
                    nc.vector.tensor_add(
                        ph[:, 0:8:4, :], ph[:, 0:8:4, :], ph[:, 2:8:4, :]
                    )
                    nc.vector.tensor_add(
                        ph[:, 0:1, :], ph[:, 0:1, :], ph[:, 4:5, :]
                    )
                    return ph

                def tree(pblk):
                    """fp16 pairwise adds: 16 P^T tiles -> 1, for the ones-mm.
                    Adjacent-pair levels (strided views) so l1a only waits on
                    the first half of the exp stream -- shortens the critical
                    path into the final drain's denominator."""
                    return tree_rest(tree_l1a(pblk), pblk)

                prev = None  # (h, si, pblk, pat, ph)
                ochunks = iter(())
                for si in range(NSB):
                    for h in range(NH_L):
                        j = h // GROUP
                        s0 = si * ST
                        pblk = pp.tile([P, TT, ST], f16, tag="pblk")
                        if prev is not None:
                            ph_prev = tree(prev[2])  # DVE, runs during stage
                            pat_prev = patp.tile([P, ST], f32, tag="pat")
                        for g in range(TT // 2):
                            psc = pscp.tile([P, 2, ST], f32, tag="psc")
                            for u in range(2):
                                tt = 2 * g + u
                                nc.tensor.matmul(
                                    psc[:, u, :],
                                    lhsT=k_sb[:, j, tt * P : (tt + 1) * P],
                                    rhs=q_sb[:, h, s0 : s0 + ST],
                                    start=True,
                                    stop=True,
                                )
                            nc.scalar.activation(
                                out=pblk[:, 2 * g : 2 * g + 2, :],
                                in_=psc,
                                func=EXP,
                                scale=SCALE,
                            )
                            if prev is not None:
                                pj = prev[0] // GROUP
                                for u in range(2):
                                    tt = 2 * g + u
                                    nc.tensor.matmul(
                                        pat_prev,
                                        lhsT=v_sb[:, tt, pj * HD : (pj + 1) * HD],
                                        rhs=prev[2][:, tt, :],
                                        start=(tt == 0),
                                        stop=(tt == TT - 1),
                                    )
                            # o_proj chunks of block si-1 interleave here.
                            # h==0 is excluded: attnT head 7 of si-1 is only
                            # normalized after this stage's g-loop, and an
                            # o-burst emitted before that normalize would
                            # wait on PE instructions that sit LATER in the
                            # in-order PE queue (deadlock).
                            if h >= 1 and (
                                g == 1 or g == 3 or (g == 6 and h <= 2)
                            ):
                                nxt = next(ochunks, None)
                                if nxt is not None:
                                    o_burst(*nxt)
                            if g == 4 and si == NSB - 1 and h == NH_L - 1:
                                # final stage: start its own tree level 1a
                                # now so the drain's denominator chain is
                                # nearly done when the PV burst finishes
                                ph_final = tree_l1a(pblk)
                            if g == 5 and prev is not None:
                                # denominator for prev: ones-mm on the tree
                                # output (partition-sum + broadcast in one)
                                pct = pscp.tile([P, 2, ST], f32, tag="psc")
                                pcs = pct[:, 0, :]
                                nc.tensor.matmul(
                                    pcs,
                                    lhsT=ones,
                                    rhs=ph_prev[:, 0, :],
                                    start=True,
                                    stop=True,
                                )
                                rcp = rcpp.tile([P, ST], f32, tag="rcp")
                                nc.vector.reciprocal_approx_fast(out=rcp, in_=pcs)
                        if prev is not None:
                            # normalize prev head into attnT (DVE, waits on
                            # the last PV matmul just emitted above)
                            ps0 = prev[1] * ST
                            nc.vector.tensor_mul(
                                attnT[:, prev[0], ps0 : ps0 + ST], pat_prev, rcp
                            )
                        prev = (h, si, pblk)
                    if si > 0:
                        # any o chunks of block si-1 not yet drained
                        for nxt in ochunks:
                            o_burst(*nxt)
                    ochunks = o_chunks(si)

                # drain the pipeline: last stage's post + last block's o_proj
                ph_prev = tree_rest(ph_final, prev[2])
                pat_prev = patp.tile([P, ST], f32, tag="pat")
                pj = prev[0] // GROUP
                for tt in range(TT):
                    nc.tensor.matmul(
                        pat_prev,
                        lhsT=v_sb[:, tt, pj * HD : (pj + 1) * HD],
                        rhs=prev[2][:, tt, :],
                        start=(tt == 0),
                        stop=(tt == TT - 1),
                    )
                pct = pscp.tile([P, 2, ST], f32, tag="psc")
                pcs = pct[:, 0, :]
                nc.tensor.matmul(
                    pcs, lhsT=ones, rhs=ph_prev[:, 0, :], start=True, stop=True
                )
                rcp = rcpp.tile([P, ST], f32, tag="rcp")
                nc.vector.reciprocal_approx_fast(out=rcp, in_=pcs)
                ps0 = prev[1] * ST
                nc.vector.tensor_mul(
                    attnT[:, prev[0], ps0 : ps0 + ST], pat_prev, rcp
                )
                for nxt in ochunks:
                    o_burst(*nxt)

    nc.compile()
    return nc


def _get_nc():
    if "nc" not in _CACHE:
        _CACHE["nc"] = _build()
    return _CACHE["nc"]


def kernel(hidden_states, cos, sin, Wq, Wk, Wv, Wo):
    if "/opt/trn_rl_repo" not in sys.path:
        sys.path.insert(0, "/opt/trn_rl_repo")
    from concourse.bass_utils import run_bass_kernel_spmd

    hidden_states = np.asarray(hidden_states, dtype=np.float32)
    cos = np.asarray(cos, dtype=np.float32)
    sin = np.asarray(sin, dtype=np.float32)
    Wq = np.asarray(Wq, dtype=np.float32)
    Wk = np.asarray(Wk, dtype=np.float32)
    Wv = np.asarray(Wv, dtype=np.float32)
    Wo = np.asarray(Wo, dtype=np.float32)

    nc = _get_nc()
    dperm = np.concatenate(
        [np.r_[16 * q : 16 * q + 16, 64 + 16 * q : 64 + 16 * q + 16] for q in range(4)]
    )
    dsign = np.where(np.arange(HD) % 32 < 16, -1.0, 1.0).astype(np.float32)

    # pretiled host layouts: partition index first, contiguous per DMA slice
    def tile_khid(w):  # [HID, F] -> [P, KT, F]
        return np.ascontiguousarray(
            w.reshape(KT, P, w.shape[1]).transpose(1, 0, 2)
        ).astype(np.float16)

    in_maps = []
    hsT_b = [
        np.ascontiguousarray(
            hidden_states[b].T.reshape(KT, P, NSB, ST).transpose(1, 2, 0, 3)
        ).astype(np.float16)
        for b in range(B)
    ]
    cosT_b = [np.ascontiguousarray(cos[b].T[dperm]).astype(np.float16) for b in range(B)]
    sinT_b = [
        np.ascontiguousarray(sin[b].T[dperm] * dsign[:, None]).astype(np.float16)
        for b in range(B)
    ]
    for c in range(2 * B):
        b, half = c // 2, c % 2
        fq = slice(half * NH_L * HD, (half + 1) * NH_L * HD)
        fkv = slice(half * NKV_L * HD, (half + 1) * NKV_L * HD)
        wq_t = tile_khid(Wq[:, fq]).reshape(P, KT, NH_L, HD).transpose(0, 2, 1, 3)
        wq_t = wq_t[:, :, :, dperm]
        wk_t = tile_khid(Wk[:, fkv]).reshape(P, KT, NKV_L, HD).transpose(0, 2, 1, 3)
        wk_t = wk_t[:, :, :, dperm]
        wo_t = np.ascontiguousarray(
            Wo[fq, :].reshape(NH_L, P, HID).transpose(1, 0, 2)
        ).astype(np.float16)
        in_maps.append(
            {
                "hsT": hsT_b[b],
                "wq": np.ascontiguousarray(wq_t),
                "wk": np.ascontiguousarray(wk_t),
                "wv": tile_khid(Wv[:, fkv]),
                "wo": wo_t,
                "cosT": cosT_b[b],
                "sinT": sinT_b[b],
            }
        )

    res = run_bass_kernel_spmd(nc, in_maps, list(range(2 * B)))
    _CACHE["last_results"] = res

    out = np.empty((B, S, HID), dtype=np.float32)
    for b in range(B):
        out[b] = res.results[2 * b]["out"] + res.results[2 * b + 1]["out"]
    return out


# revision 30
# speedup vs baseline: 1.0052x; 1.0052x over previous
"""GQA attention (RoPE + softmax + o_proj) on 8 Trainium2 NeuronCores.

Problem shapes (hardcoded): hidden_states [4, 2048, 2048], 16 q heads,
4 kv heads, head_dim 128, rope cos/sin tables given as inputs.

Sharding: core c -> (batch b = c // 2, q-head half = c % 2).  Each core
computes 8 q heads + their 2 kv heads for one batch and produces a
partial o_proj output [2048, 2048]; the host sums the two halves per
batch (tensor parallel, no device collectives).

All matmuls run in fp16 (1 cycle/row on PE) with fp32 PSUM accumulation.
Key engine-level structure (from perfetto trace analysis):
  - 512-wide moving dims everywhere so LoadStationary (97ns) hides
    behind the 213ns matmul; v is therefore computed TRANSPOSED
    (wv-halves stationary, hs moving 512) at full rate and rotated back
    into [t, d] layout with per-tile DMA crossbar transposes on the
    sync queue -- zero PE/DVE cost (the old [t,d]-direct v projection
    had a 256-wide moving dim and ran at half rate, ~27us lost).
  - RoPE via a partition shuffle (host-permuted head dim) on DVE.
  - scores^T[t, s] with k^T tiles stationary; exp via ScalarE (fused
    1/sqrt(d) scale) reads two PSUM banks per instruction and writes
    P^T fp16 straight to SBUF.
  - phase B software pipeline: stage (h,si) emits its 8 score pairs
    interleaved with the PREVIOUS stage's PV pairs and with o_proj
    matmul bursts of the previous s-block, so PE never idles while
    ScalarE's exp stream (~8us/stage) drains.
  - softmax denominators: four fp16 DVE pairwise adds over P^T down to
    one tile, then a single all-ones stationary matmul (result
    replicated across partitions = pre-broadcast), fast DVE reciprocal,
    fused normalize+cast on the attn PSUM->SBUF copyback.
  - PSUM: score-pair pool (3 bufs x 2 banks) also serves the o_proj and
    denominator accumulators (short-lived allocs in rotation); PV
    accumulators get 2 dedicated banks.
"""

import sys

import numpy as np

B, S, HID = 4, 2048, 2048
NH, NKV, HD = 16, 4, 128
NH_L = 8        # q heads per core
NKV_L = 2       # kv heads per core
GROUP = NH // NKV
P = 128
ST = 512        # s-block (matmul free dim)
NSB = S // ST   # 4 s-blocks
KT = HID // P   # 16 contraction tiles over hidden
TT = S // P     # 16 key/t tiles
SCALE = 1.0 / float(np.sqrt(HD))

_CACHE = {}


def _build():
    if "/opt/trn_rl_repo" not in sys.path:
        sys.path.insert(0, "/opt/trn_rl_repo")
    import concourse.mybir as mybir
    from concourse import bacc
    from concourse.tile import TileContext
    from concourse.tile_rust import add_dep_helper

    dt = mybir.dt
    f16, f32 = dt.float16, dt.float32

    nc = bacc.Bacc("TRN2", target_bir_lowering=False, debug=False, num_devices=8)
    # host-pretiled layouts (see kernel() below)
    hsT = nc.dram_tensor("hsT", [P, NSB, KT, ST], f16, kind="ExternalInput").ap()
    wq = nc.dram_tensor("wq", [P, NH_L, KT, HD], f16, kind="ExternalInput").ap()
    wk = nc.dram_tensor("wk", [P, NKV_L, KT, HD], f16, kind="ExternalInput").ap()
    wv = nc.dram_tensor("wv", [P, KT, NKV_L * HD], f16, kind="ExternalInput").ap()
    wo = nc.dram_tensor("wo", [P, NH_L, HID], f16, kind="ExternalInput").ap()
    cosT = nc.dram_tensor("cosT", [HD, S], f16, kind="ExternalInput").ap()
    sinT = nc.dram_tensor("sinT", [HD, S], f16, kind="ExternalInput").ap()
    out = nc.dram_tensor("out", [S, HID], f32, kind="ExternalOutput").ap()

    EXP = mybir.ActivationFunctionType.Exp

    with TileContext(nc) as tc:
        with (
            tc.tile_pool(name="consts", bufs=1) as consts,
            tc.tile_pool(name="qkv", bufs=1) as qkvp,
        ):
            ones = consts.tile([P, P], f16, tag="ones")
            nc.vector.memset(ones, 1.0)
            # rotate_half as an intra-quadrant partition shuffle (the head
            # dim is host-permuted so +-64 pairs sit 16 apart per quadrant;
            # the sign lives in the pre-negated sin table)
            SHUF = list(range(16, 32)) + list(range(0, 16))

            q_sb = qkvp.tile([P, NH_L, S], f16, tag="q")
            k_sb = qkvp.tile([P, NKV_L, S], f16, tag="k")
            v_sb = qkvp.tile([P, TT, NKV_L * HD], f16, tag="v")

            # ---------------- Phase A: projections + RoPE ----------------
            with (
                tc.tile_pool(name="wqkv", bufs=1) as wp,
                tc.tile_pool(name="trig", bufs=1) as trig,
                tc.tile_pool(name="vT", bufs=1) as vtp,
                tc.tile_pool(name="hs", bufs=2) as hsp,
                tc.tile_pool(name="ropes", bufs=4) as smalls,
                tc.tile_pool(name="psA", bufs=6, space="PSUM") as psA,
            ):
                vT_sb = vtp.tile([P, NKV_L, S], f16, tag="vT")
                # hs block 0 first (its consumers are the head of the program)
                hs_blks = {}
                # first hs block + wv arrive in interleaved chunks so the
                # very first projection group starts after ~0.5MB, not ~3MB
                hs_first = hsp.tile([P, KT, ST], f16, tag="hs")
                wv_sb = wp.tile([P, KT, NKV_L * HD], f16, tag="wv")
                # hs block 0 split across two queues (scalar+gpsimd) so the
                # first projection group is fed at ~2x single-queue rate
                # fine-grained alternating chunks on two dedicated queues so
                # the first vT/k projections are fed smoothly; everything not
                # needed in the first ~15us (wk, cos/sin, wq, wo) is deferred
                # behind hs0 so the critical 3MB (hs block 0 + wv) gets the
                # full ~358 GB/s of per-core HBM bandwidth
                nc.scalar.dma_start(out=hs_first[:, 0:1, :], in_=hsT[:, 0, 0:1, :])
                nc.sync.dma_start(out=wv_sb[:, 0:2, :], in_=wv[:, 0:2, :])
                nc.gpsimd.dma_start(out=hs_first[:, 1:3, :], in_=hsT[:, 0, 1:3, :])
                nc.scalar.dma_start(out=hs_first[:, 3:5, :], in_=hsT[:, 0, 3:5, :])
                nc.sync.dma_start(out=wv_sb[:, 2:9, :], in_=wv[:, 2:9, :])
                nc.gpsimd.dma_start(out=hs_first[:, 5:7, :], in_=hsT[:, 0, 5:7, :])
                nc.scalar.dma_start(out=hs_first[:, 7:9, :], in_=hsT[:, 0, 7:9, :])
                nc.gpsimd.dma_start(out=hs_first[:, 9:11, :], in_=hsT[:, 0, 9:11, :])
                nc.scalar.dma_start(out=hs_first[:, 11:13, :], in_=hsT[:, 0, 11:13, :])
                nc.sync.dma_start(out=wv_sb[:, 9:16, :], in_=wv[:, 9:16, :])
                hs0_dma = nc.gpsimd.dma_start(
                    out=hs_first[:, 13:16, :], in_=hsT[:, 0, 13:16, :]
                )
                hs_blks[0] = hs_first
                hs_dmas = [hs0_dma]

                # wk rides the scalar queue behind its hs chunks: starts after
                # ~0.9MB of hs, so kv-head 0's slice lands right as the first
                # k-projection needs it (~17us) without competing earlier
                wk_sb = wp.tile([P, NKV_L, KT, HD], f16, tag="wk")
                nc.scalar.dma_start(out=wk_sb[:, 0], in_=wk[:, 0])
                nc.scalar.dma_start(out=wk_sb[:, 1], in_=wk[:, 1])

                wq_sb = wp.tile([P, NH_L, KT, HD], f16, tag="wq")
                nc.sync.dma_start(out=wq_sb[:, 0, :, :], in_=wq[:, 0, :, :])

                cos_sb = trig.tile([HD, S], f16, tag="cos")
                nc.sync.dma_start(out=cos_sb, in_=cosT)
                sin_sb = trig.tile([HD, S], f16, tag="sin")
                nc.sync.dma_start(out=sin_sb, in_=sinT)
                for h in range(1, NH_L):  # per-head DMAs so early heads land first
                    nc.sync.dma_start(out=wq_sb[:, h, :, :], in_=wq[:, h, :, :])

                # software pipeline: the rot-shuffle + rope combine for one
                # projection is emitted while the NEXT projection's matmul
                # group runs, so PE never waits on the PSUM copyback.
                pending = []

                def rope_flush():
                    qc, s0, dst, dsti = pending.pop(0)
                    rc = smalls.tile([P, ST], f16, tag="rc")
                    nc.vector.stream_shuffle(rc, qc, SHUF)
                    t1 = smalls.tile([P, ST], f16, tag="t1")
                    nc.vector.tensor_mul(t1, qc, cos_sb[:, s0 : s0 + ST])
                    t2 = smalls.tile([P, ST], f16, tag="t2")
                    nc.vector.tensor_mul(t2, rc, sin_sb[:, s0 : s0 + ST])
                    nc.vector.tensor_add(dst[:, dsti, s0 : s0 + ST], t1, t2)

                for si in range(NSB):
                    s0 = si * ST
                    if si in hs_blks:
                        hs_blk = hs_blks[si]
                    else:
                        hs_blk = hsp.tile([P, KT, ST], f16, tag="hs")
                        # gpsimd queue is otherwise idle, so chaining these
                        # issues behind the previous block stalls nothing
                        hd = nc.gpsimd.dma_start(out=hs_blk, in_=hsT[:, si, :, :])
                        add_dep_helper(
                            hd.ins,
                            hs_dmas[-1].ins,
                            sync=True,
                            reason="stagger hs blocks",
                        )
                        hs_dmas.append(hd)

                    def proj(w_slice, dst, dsti):
                        pm = psA.tile([P, ST], f32, tag="ps")
                        for kt in range(KT):
                            nc.tensor.matmul(
                                pm,
                                lhsT=w_slice[:, kt, :],
                                rhs=hs_blk[:, kt, :],
                                start=(kt == 0),
                                stop=(kt == KT - 1),
                            )
                        qc = smalls.tile([P, ST], f16, tag="qc")
                        nc.vector.tensor_copy(qc, pm)
                        pending.append((qc, s0, dst, dsti))

                    # v^T first: needs only hs + the small wv.  Full-rate
                    # (512-wide moving): wv column-half stationary, hs moving.
                    for j in range(NKV_L):
                        pv = psA.tile([P, ST], f32, tag="ps")
                        for kt in range(KT):
                            nc.tensor.matmul(
                                pv,
                                lhsT=wv_sb[:, kt, j * HD : (j + 1) * HD],
                                rhs=hs_blk[:, kt, :],
                                start=(kt == 0),
                                stop=(kt == KT - 1),
                            )
                        nc.scalar.copy(vT_sb[:, j, s0 : s0 + ST], pv)
                    for jk in range(NKV_L):
                        proj(wk_sb[:, jk], k_sb, jk)
                        if len(pending) > 1:
                            rope_flush()
                    # rotate v^T[d, s] back into v[t, d] tiles for PV via the
                    # DMA crossbar transpose (zero PE/DVE cost; the gpsimd
                    # queue is otherwise idle).  Per-tile calls: the dst of
                    # each is a contiguous 128-elem run per partition (3D
                    # strided xbar dsts are a known-wrong path on HW).
                    for sj in range(ST // P):
                        tt = si * (ST // P) + sj
                        for j in range(NKV_L):
                            nc.sync.dma_start_transpose(
                                v_sb[:, tt, j * HD : (j + 1) * HD],
                                vT_sb[:, j, s0 + sj * P : s0 + (sj + 1) * P],
                            )
                    for h in range(NH_L):
                        proj(wq_sb[:, h], q_sb, h)
                        if len(pending) > 1:
                            rope_flush()
                while pending:
                    rope_flush()

            # ---------------- Phase B: attention + interleaved o_proj ------
            with (
                tc.tile_pool(name="wo", bufs=1) as wop,
                tc.tile_pool(name="attn", bufs=1) as ap_,
                tc.tile_pool(name="pblk", bufs=2) as pp,
                tc.tile_pool(name="phalf", bufs=2) as php,
                tc.tile_pool(name="rcps", bufs=4) as rcpp,
                tc.tile_pool(name="outp", bufs=3) as op_,
                tc.tile_pool(name="psc", bufs=3, space="PSUM") as pscp,
                tc.tile_pool(name="pat", bufs=2, space="PSUM") as patp,
            ):
                wo_sb = wop.tile([P, NH_L, HID], f16, tag="wo")
                wod = nc.sync.dma_start(out=wo_sb, in_=wo)
                add_dep_helper(
                    wod.ins, hs0_dma.ins, sync=True, reason="defer wo behind hs0"
                )
                attnT = ap_.tile([P, NH_L, S], f16, tag="attnT")
                QT = TT // 4

                def o_burst(si, sj, ni):
                    """one o_proj psum group: 8 matmuls + copyback + DMA.
                    Copyback on DVE: ScalarE runs at ~98% on the exp stream."""
                    st = si * (ST // P) + sj
                    pot = pscp.tile([P, 2, ST], f32, tag="psc")
                    po = pot[:, 0, :]
                    for ft in range(NH_L):
                        nc.tensor.matmul(
                            po,
                            lhsT=attnT[:, ft, st * P : (st + 1) * P],
                            rhs=wo_sb[:, ft, ni * ST : (ni + 1) * ST],
                            start=(ft == 0),
                            stop=(ft == NH_L - 1),
                        )
                    ob = op_.tile([P, ST], f32, tag="ob")
                    nc.vector.tensor_copy(ob, po)
                    nc.sync.dma_start(
                        out=out[st * P : (st + 1) * P, ni * ST : (ni + 1) * ST],
                        in_=ob,
                    )

                def o_chunks(si):
                    for sj in range(ST // P):
                        for ni in range(HID // ST):
                            yield (si, sj, ni)

                def tree_l1a(pblk):
                    """first tree level over P^T tiles 0-7 (exp groups 0-3)"""
                    ph = php.tile([P, TT // 2, ST], f16, tag="ph")
                    nc.vector.tensor_add(
                        ph[:, 0:4, :], pblk[:, 0:8:2, :], pblk[:, 1:8:2, :]
                    )
                    return ph

                def tree_l1b1(ph, pblk):
                    """tree level 1 over P^T tiles 8-11 (exp groups 4-5)"""
                    nc.vector.tensor_add(
                        ph[:, 4:6, :], pblk[:, 8:12:2, :], pblk[:, 9:12:2, :]
                    )

                def tree_rest(ph, pblk):
                    nc.vector.tensor_add(
                        ph[:, 6:8, :], pblk[:, 12:16:2, :], pblk[:, 13:16:2, :]
                    )
                    nc.vector.tensor_add(
                        ph[:, 0:8:2, :], ph[:, 0:8:2, :], ph[:, 1:8:2, :]
                    )
                    nc.vector.tensor_add(
                        ph[:, 0:8:4, :], ph[:, 0:8:4, :], ph[:, 2:8:4, :]
                    )
                    nc.vector.tensor_add(
                        ph[:, 0:1, :], ph[:, 0:1, :], ph[:, 4:5, :]
                    )
                    return ph

                def tree(pblk):
                    """fp16 pairwise adds: 16 P^T tiles -> 1, for the ones-mm.
                    Adjacent-pair levels (strided views) so l1a only waits on
                    the first half of the exp stream -- shortens the critical
                    path into the final drain's denominator."""
                    ph = tree_l1a(pblk)
                    tree_l1b1(ph, pblk)
                    return tree_rest(ph, pblk)

                prev = None  # (h, si, pblk, pat, ph)
                ochunks = iter(())
                for si in range(NSB):
                    for h in range(NH_L):
                        j = h // GROUP
                        s0 = si * ST
                        pblk = pp.tile([P, TT, ST], f16, tag="pblk")
                        if prev is not None:
                            ph_prev = tree(prev[2])  # DVE, runs during stage
                            pat_prev = patp.tile([P, ST], f32, tag="pat")
                        for g in range(TT // 2):
                            psc = pscp.tile([P, 2, ST], f32, tag="psc")
                            for u in range(2):
                                tt = 2 * g + u
                                nc.tensor.matmul(
                                    psc[:, u, :],
                                    lhsT=k_sb[:, j, tt * P : (tt + 1) * P],
                                    rhs=q_sb[:, h, s0 : s0 + ST],
                                    start=True,
                                    stop=True,
                                )
                            nc.scalar.activation(
                                out=pblk[:, 2 * g : 2 * g + 2, :],
                                in_=psc,
                                func=EXP,
                                scale=SCALE,
                            )
                            if prev is not None:
                                pj = prev[0] // GROUP
                                for u in range(2):
                                    tt = 2 * g + u
                                    nc.tensor.matmul(
                                        pat_prev,
                                        lhsT=v_sb[:, tt, pj * HD : (pj + 1) * HD],
                                        rhs=prev[2][:, tt, :],
                                        start=(tt == 0),
                                        stop=(tt == TT - 1),
                                    )
                            # o_proj chunks of block si-1 interleave here.
                            # h==0 is excluded: attnT head 7 of si-1 is only
                            # normalized after this stage's g-loop, and an
                            # o-burst emitted before that normalize would
                            # wait on PE instructions that sit LATER in the
                            # in-order PE queue (deadlock).
                            if h >= 1 and (
                                g == 1 or g == 3 or (g == 6 and h <= 2)
                            ):
                                nxt = next(ochunks, None)
                                if nxt is not None:
                                    o_burst(*nxt)
                            if g == 4 and si == NSB - 1 and h == NH_L - 1:
                                # final stage: start its own tree level 1a
                                # now so the drain's denominator chain is
                                # nearly done when the PV burst finishes
                                ph_final = tree_l1a(pblk)
                            if g == 7 and si == NSB - 1 and h == NH_L - 1:
                                tree_l1b1(ph_final, pblk)
                            if g == 5 and prev is not None:
                                # denominator for prev: ones-mm on the tree
                                # output (partition-sum + broadcast in one)
                                pct = pscp.tile([P, 2, ST], f32, tag="psc")
                                pcs = pct[:, 0, :]
                                nc.tensor.matmul(
                                    pcs,
                                    lhsT=ones,
                                    rhs=ph_prev[:, 0, :],
                                    start=True,
                                    stop=True,
                                )
                                rcp = rcpp.tile([P, ST], f32, tag="rcp")
                                nc.vector.reciprocal_approx_fast(out=rcp, in_=pcs)
                        if prev is not None:
                            # normalize prev head into attnT (DVE, waits on
                            # the last PV matmul just emitted above)
                            ps0 = prev[1] * ST
                            nc.vector.tensor_mul(
                                attnT[:, prev[0], ps0 : ps0 + ST], pat_prev, rcp
                            )
                        prev = (h, si, pblk)
                    if si > 0:
                        # any o chunks of block si-1 not yet drained
                        for nxt in ochunks:
                            o_burst(*nxt)
                    ochunks = o_chunks(si)

                # drain the pipeline: last stage's post + last block's o_proj
                ph_prev = tree_rest(ph_final, prev[2])
                pat_prev = patp.tile([P, ST], f32, tag="pat")
                pj = prev[0] // GROUP
                for tt in range(TT):
                    nc.tensor.matmul(
                        pat_prev,
                        lhsT=v_sb[:, tt, pj * HD : (pj + 1) * HD],
                        rhs=prev[2][:, tt, :],
                        start=(tt == 0),
                        stop=(tt == TT - 1),
                    )
                pct = pscp.tile([P, 2, ST], f32, tag="psc")
                pcs = pct[:, 0, :]
                nc.tensor.matmul(
                    pcs, lhsT=ones, rhs=ph_prev[:, 0, :], start=True, stop=True
                )
                rcp = rcpp.tile([P, ST], f32, tag="rcp")
                nc.vector.reciprocal_approx_fast(out=rcp, in_=pcs)
                ps0 = prev[1] * ST
                nc.vector.tensor_mul(
                    attnT[:, prev[0], ps0 : ps0 + ST], pat_prev, rcp
                )
                for nxt in ochunks:
                    o_burst(*nxt)

    nc.compile()
    return nc


def _get_nc():
    if "nc" not in _CACHE:
        _CACHE["nc"] = _build()
    return _CACHE["nc"]


def kernel(hidden_states, cos, sin, Wq, Wk, Wv, Wo):
    if "/opt/trn_rl_repo" not in sys.path:
        sys.path.insert(0, "/opt/trn_rl_repo")
    from concourse.bass_utils import run_bass_kernel_spmd

    hidden_states = np.asarray(hidden_states, dtype=np.float32)
    cos = np.asarray(cos, dtype=np.float32)
    sin = np.asarray(sin, dtype=np.float32)
    Wq = np.asarray(Wq, dtype=np.float32)
    Wk = np.asarray(Wk, dtype=np.float32)
    Wv = np.asarray(Wv, dtype=np.float32)
    Wo = np.asarray(Wo, dtype=np.float32)

    nc = _get_nc()
    dperm = np.concatenate(
        [np.r_[16 * q : 16 * q + 16, 64 + 16 * q : 64 + 16 * q + 16] for q in range(4)]
    )
    dsign = np.where(np.arange(HD) % 32 < 16, -1.0, 1.0).astype(np.float32)

    # pretiled host layouts: partition index first, contiguous per DMA slice
    def tile_khid(w):  # [HID, F] -> [P, KT, F]
        return np.ascontiguousarray(
            w.reshape(KT, P, w.shape[1]).transpose(1, 0, 2)
        ).astype(np.float16)

    in_maps = []
    hsT_b = [
        np.ascontiguousarray(
            hidden_states[b].T.reshape(KT, P, NSB, ST).transpose(1, 2, 0, 3)
        ).astype(np.float16)
        for b in range(B)
    ]
    cosT_b = [np.ascontiguousarray(cos[b].T[dperm]).astype(np.float16) for b in range(B)]
    sinT_b = [
        np.ascontiguousarray(sin[b].T[dperm] * dsign[:, None]).astype(np.float16)
        for b in range(B)
    ]
    for c in range(2 * B):
        b, half = c // 2, c % 2
        fq = slice(half * NH_L * HD, (half + 1) * NH_L * HD)
        fkv = slice(half * NKV_L * HD, (half + 1) * NKV_L * HD)
        wq_t = tile_khid(Wq[:, fq]).reshape(P, KT, NH_L, HD).transpose(0, 2, 1, 3)
        wq_t = wq_t[:, :, :, dperm]
        wk_t = tile_khid(Wk[:, fkv]).reshape(P, KT, NKV_L, HD).transpose(0, 2, 1, 3)
        wk_t = wk_t[:, :, :, dperm]
        wo_t = np.ascontiguousarray(
            Wo[fq, :].reshape(NH_L, P, HID).transpose(1, 0, 2)
        ).astype(np.float16)
        in_maps.append(
            {
                "hsT": hsT_b[b],
                "wq": np.ascontiguousarray(wq_t),
                "wk": np.ascontiguousarray(wk_t),
                "wv": tile_khid(Wv[:, fkv]),
                "wo": wo_t,
                "cosT": cosT_b[b],
                "sinT": sinT_b[b],
            }
        )

    res = run_bass_kernel_spmd(nc, in_maps, list(range(2 * B)))
    _CACHE["last_results"] = res

    out = np.empty((B, S, HID), dtype=np.float32)
    for b in range(B):
        out[b] = res.results[2 * b]["out"] + res.results[2 * b + 1]["out"]
    return out
